# revision 1
# baseline (speedup 1.0000x reference)
"""Trainium2 Bass kernel for the BottleneckBlock (conv -> IN -> lrelu -> self-attn
-> conv -> IN -> +residual -> lrelu), data-parallel over batch across 8 cores:
each NeuronCore computes one batch element end to end (no collectives).

Per-core layout: channels on partitions, length L on the free dimension.
Convs/QK run in bf16 (fp32 PSUM accumulation); the P@V and softmax-Z matmuls
run in fp8e4m3 with MatmulPerfMode.DoubleRow (2x PE throughput; softmax
renormalization makes the fp8 quantization of P/v numerically free).  v is
produced directly transposed (vT = hT @ wvT) so P@V needs no PE transposes.
Z is reduced over partitions with a ones-column matmul, inverted on DVE, and
broadcast back on GPSIMD.  b1/b2 are dropped: InstanceNorm cancels per-channel
constant biases.  IN-apply fuses normalize (ACT Identity w/ per-partition
scale+bias) and leaky-relu (one DVE scalar_tensor_tensor max(x, 0.2x)).

Inputs are packed into three DRAM tensors (w1 / everything-else / x) issued on
three different DMA rings (SP / SWDGE / ACT): one completion semaphore each —
walrus allows only ~1-2 sync-waits per instruction (Bacc splits the rest onto
NOPs) and big packed transfers run at full 16-engine SDMA bandwidth.
"""
import numpy as np
import ml_dtypes

import concourse.bass as bass
import concourse.bacc as bacc
import concourse.mybir as mybir
import concourse.tile as tile
from concourse.bass_utils import run_bass_kernel_spmd

DT = mybir.dt
ALU = mybir.AluOpType
AF = mybir.ActivationFunctionType
BF16 = ml_dtypes.bfloat16

B, C, L = 8, 256, 2048
CR, CO, KW = 32, 512, 5
PAD = KW // 2
LP = L + 2 * PAD          # padded length
NCH = L // 512            # 512-wide l-chunks
NMT = L // 128            # 128-wide m-tiles
SCALE = CR ** (-0.5)
EPS = 1e-5
SLOPE = 0.2

# packed-weights segment offsets (elements per partition, bf16)
# pack A holds only w1t (so conv1 can start as soon as it lands);
# pack B holds everything else.
_SEG = {}
_off = 0
for _name, _sz in (("w2t", 2 * KW * CO),
                   ("wqt", 2 * CR), ("wkt", 2 * CR), ("wvt", 2 * C),
                   ("wot", 2 * C), ("wrt", 2 * CO), ("ones_col", 1),
                   ("bq_col", 1), ("bk_col", 1), ("bo_col", 2), ("br_col", 4)):
    _SEG[_name] = (_off, _off + _sz)
    _off += _sz
# row-0 segments (v-bias row + ones row)
for _name, _sz in (("bv", C), ("ones_row", 512)):
    _SEG[_name] = (_off, _off + _sz)
    _off += _sz
F_PACK = _off
F_W1 = 2 * KW * C

_CACHED_NC = None


def _build():
    nc = bacc.Bacc("TRN2", target_bir_lowering=False)

    x_d = nc.dram_tensor("x", [128, 2, LP], DT.bfloat16, kind="ExternalInput")
    w1_d = nc.dram_tensor("w1pack", [128, F_W1], DT.bfloat16, kind="ExternalInput")
    wp_d = nc.dram_tensor("wpack", [128, F_PACK], DT.bfloat16, kind="ExternalInput")
    out_d = nc.dram_tensor("out", [CO, L], DT.float32, kind="ExternalOutput")

    with tile.TileContext(nc) as tc:
        with (
            tc.tile_pool(name="consts", bufs=1) as consts,
            tc.tile_pool(name="big", bufs=1) as big,
            tc.tile_pool(name="ptp", bufs=3) as ptp,
            tc.tile_pool(name="stat", bufs=2) as statp,
            tc.tile_pool(name="small", bufs=8) as smallp,
            tc.tile_pool(name="tmp", bufs=6) as tmpp,
            tc.tile_pool(name="outp", bufs=6) as outp,
        ):
            w1all = consts.tile([128, F_W1], DT.bfloat16, tag="w1all")
            nc.sync.dma_start(out=w1all[:, 0:F_W1 // 2], in_=w1_d[:, 0:F_W1 // 2])
            nc.sync.dma_start(out=w1all[:, F_W1 // 2:], in_=w1_d[:, F_W1 // 2:])
            xall0 = big.tile([128, 2, LP], DT.bfloat16, tag="xall")
            for _a, _b in ((0, 516), (516, 1028), (1028, LP)):
                nc.scalar.dma_start(out=xall0[:, :, _a:_b], in_=x_d[:, :, _a:_b])
            wall = consts.tile([128, F_PACK], DT.bfloat16, tag="wall")
            nc.gpsimd.dma_start(out=wall, in_=wp_d[:, :])

            def seg(name):
                a, b = _SEG[name]
                return wall[:, a:b]

            w1t = w1all[:, :].rearrange("p (i k o) -> p i k o", i=2, k=KW)
            w2t = seg("w2t").rearrange("p (i k o) -> p i k o", i=2, k=KW)
            wqt = seg("wqt").rearrange("p (i o) -> p i o", i=2)
            wkt = seg("wkt").rearrange("p (i o) -> p i o", i=2)
            wvt = seg("wvt").rearrange("p (i o) -> p i o", i=2)
            wot = seg("wot").rearrange("p (i o) -> p i o", i=2)
            wrt = seg("wrt").rearrange("p (i o) -> p i o", i=2)
            ones_col = seg("ones_col")
            bq_col = seg("bq_col")
            bk_col = seg("bk_col")
            bo_col = seg("bo_col")
            br_col = seg("br_col")
            bv = seg("bv")[0:1]
            ones_row = seg("ones_row")[0:1]
            ones_bf = ones_row[:, 0:128]

            eps_t = consts.tile([128, 1], DT.float32, tag="eps")
            nc.vector.memset(eps_t, EPS)
            ones8p = consts.tile([128, 2, 16], DT.float8e4, tag="ones8p")
            nc.vector.memset(ones8p, 1.0)
            ones8 = ones8p[:, :, 0:1]
            bqf = consts.tile([32, 1], DT.float32, tag="bqf")
            nc.vector.tensor_copy(bqf, bq_col[0:32])
            bkf = consts.tile([32, 1], DT.float32, tag="bkf")
            nc.vector.tensor_copy(bkf, bk_col[0:32])
            bof = consts.tile([128, 2], DT.float32, tag="bof")
            nc.vector.tensor_copy(bof, bo_col)
            brf = consts.tile([128, 4], DT.float32, tag="brf")
            nc.vector.tensor_copy(brf, br_col)

            # ---------------- persistent activations ----------------
            xall = xall0
            xp = [xall[:, i, :] for i in range(2)]
            hp = [big.tile([128, LP], DT.bfloat16, tag=f"hp{i}", name=f"hp{i}")
                  for i in range(2)]
            h2p = [big.tile([128, LP], DT.bfloat16, tag=f"h2p{i}", name=f"h2p{i}")
                   for i in range(2)]
            for i in range(2):
                for t in (hp[i], h2p[i]):
                    nc.vector.memset(t[:, 0:PAD], 0.0)
                    nc.vector.memset(t[:, LP - PAD:LP], 0.0)
            qs = big.tile([32, L], DT.bfloat16, tag="qs")
            ks = big.tile([32, L], DT.bfloat16, tag="ks")
            vT = big.tile([128, NMT, C], DT.float8e4, tag="vT")
            os_ = [big.tile([128, L], DT.bfloat16, tag=f"os{i}", name=f"os{i}")
                   for i in range(2)]

            def mm(p, lhsT, rhs, first, last, pm=None):
                nc.tensor.matmul(p, lhsT=lhsT, rhs=rhs, start=first, stop=last,
                                 perf_mode=pm)

            # ---------------- conv1 + instance norm + leaky ----------------
            with tc.tile_pool(name="psA", bufs=8, space="PSUM") as psA:
                for t in range(2):
                    osl = slice(t * 128, (t + 1) * 128)
                    st = statp.tile([128, NCH, 6], DT.float32, tag="st1")
                    chunks = []
                    for lc in range(NCH):
                        p = psA.tile([128, 512], DT.float32, tag="a",
                                     name=f"c1p{t}{lc}")
                        n = 0
                        for i in range(2):
                            for k in range(KW):
                                mm(p, w1t[:, i, k, osl],
                                   xp[i][:, lc * 512 + k: lc * 512 + k + 512],
                                   n == 0, n == 9)
                                n += 1
                        nc.vector.bn_stats(out=st[:, lc, :], in_=p)
                        chunks.append(p)
                    mv = smallp.tile([128, 2], DT.float32, tag="mv")
                    rstd = smallp.tile([128, 1], DT.float32, tag="rstd")
                    negm = smallp.tile([128, 1], DT.float32, tag="negm")
                    nc.vector.bn_aggr(out=mv, in_=st)
                    nc.scalar.activation(out=rstd, in_=mv[:, 1:2], func=AF.Sqrt,
                                         bias=eps_t, scale=1.0)
                    nc.vector.reciprocal(out=rstd, in_=rstd)
                    nc.vector.tensor_scalar(out=negm, in0=mv[:, 0:1], scalar1=rstd,
                                            scalar2=-1.0, op0=ALU.mult, op1=ALU.mult)
                    for lc in range(NCH):
                        tmp = tmpp.tile([128, 512], DT.float32, tag="tmp")
                        if t == 1 and lc < 2:
                            nc.vector.tensor_scalar(out=tmp, in0=chunks[lc],
                                                    scalar1=rstd, scalar2=negm,
                                                    op0=ALU.mult, op1=ALU.add)
                        else:
                            nc.scalar.activation(out=tmp, in_=chunks[lc],
                                                 func=AF.Identity, bias=negm,
                                                 scale=rstd)
                        nc.vector.scalar_tensor_tensor(
                            out=hp[t][:, PAD + lc * 512:PAD + (lc + 1) * 512],
                            in0=tmp, scalar=SLOPE, in1=tmp, op0=ALU.mult, op1=ALU.max)
                    last_rstd = rstd

                # prefetch exp act-table while PE is busy with q/k/vT
                dummy = smallp.tile([1, 1], DT.float32, tag="dummy")
                nc.scalar.activation(out=dummy, in_=last_rstd[0:1, :], func=AF.Exp,
                                     scale=1.0)

                # ---- q, k + vT interleaved by hp-chunk dependency so PE
                # ---- can start as soon as the first hp chunk is applied
                for lc in range(NCH):
                    lsl = slice(PAD + lc * 512, PAD + lc * 512 + 512)
                    for dst, wt, bias in ((qs, wqt, bqf), (ks, wkt, bkf)):
                        p = psA.tile([32, 512], DT.float32, tag="a",
                                     name=f"qk{lc}")
                        mm(p, wt[:, 0, :], hp[0][:, lsl], True, False)
                        mm(p, wt[:, 1, :], hp[1][:, lsl], False, True)
                        if lc % 2 == 0:
                            nc.vector.tensor_scalar(
                                out=dst[:, lc * 512:(lc + 1) * 512], in0=p,
                                scalar1=bias, scalar2=None, op0=ALU.add)
                        else:
                            nc.scalar.activation(
                                out=dst[:, lc * 512:(lc + 1) * 512], in_=p,
                                func=AF.Identity, bias=bias, scale=1.0)
                    for mt in (4 * lc, 4 * lc + 2):
                        p = psA.tile([128, 2, C], DT.float32, tag="a",
                                     name=f"vt{mt}")
                        for j in range(2):
                            msl = slice(PAD + (mt + j) * 128,
                                        PAD + (mt + j) * 128 + 128)
                            mm(p[:, j, :], hp[0][:, msl], wvt[:, 0, :], True, False)
                            mm(p[:, j, :], hp[1][:, msl], wvt[:, 1, :], False, False)
                            mm(p[:, j, :], ones_bf, bv, False, True)
                        if mt % 4 == 0:
                            nc.vector.tensor_copy(vT[:, mt:mt + 2, :], p)
                        else:
                            nc.scalar.copy(out=vT[:, mt:mt + 2, :], in_=p)

            # ---------------- attention per l-chunk ----------------
            with (
                tc.tile_pool(name="psw", bufs=1, space="PSUM") as psw,
                tc.tile_pool(name="ps2", bufs=2, space="PSUM") as ps2,
                tc.tile_pool(name="psacc", bufs=1, space="PSUM") as psacc,
                tc.tile_pool(name="psz", bufs=1, space="PSUM") as psz,
            ):
              for lc in range(NCH):
                  lsl = slice(lc * 512, (lc + 1) * 512)
                  pt = ptp.tile([128, NMT, 512], DT.float8e4, tag="pt")
                  po = [psacc.tile([128, 512], DT.float32, tag=f"oc{t}", name=f"oc{t}")
                        for t in range(2)]
                  pz = psz.tile([1, 512], DT.float32, tag="z")
                  for mt in range(0, NMT, 2):
                      mp = slice(mt, mt + 2)
                      ps = ps2.tile([128, 2, 512], DT.float32, tag="s2")
                      mm(ps[:, 0, :], ks[:, mt * 128:(mt + 1) * 128], qs[:, lsl],
                         True, True)
                      mm(ps[:, 1, :], ks[:, (mt + 1) * 128:(mt + 2) * 128],
                         qs[:, lsl], True, True)
                      nc.scalar.activation(out=pt[:, mp, :], in_=ps, func=AF.Exp,
                                           scale=SCALE)
                      DR = mybir.MatmulPerfMode.DoubleRow
                      for t in range(2):
                          mm(po[t], vT[:, mp, t * 128:(t + 1) * 128],
                             pt[:, mp, :], mt == 0, mt == NMT - 2, pm=DR)
                      mm(pz, ones8, pt[:, mp, :], mt == 0, mt == NMT - 2, pm=DR)
                  zrec = smallp.tile([1, 512], DT.float32, tag="zrec")
                  nc.vector.reciprocal(out=zrec, in_=pz)
                  bcs = tmpp.tile([128, 512], DT.float32, tag="bcs")
                  nc.gpsimd.partition_broadcast(bcs, zrec)
                  if lc < NCH - 1:
                      for t in range(2):
                          nc.vector.tensor_tensor(out=os_[t][:, lsl], in0=po[t],
                                                  in1=bcs, op=ALU.mult)
                  else:
                      # last chunk: let wo consume unnormalized O (1/Z commutes
                      # through the channel contraction) so its matmuls don't
                      # wait on the recip/broadcast chain
                      for t in range(2):
                          nc.scalar.copy(out=os_[t][:, lsl], in_=po[t])
                      last_bcs = bcs

                  def wo_chunk(wlc):
                      wsl = slice(wlc * 512, (wlc + 1) * 512)
                      for t in range(2):
                          osl = slice(t * 128, (t + 1) * 128)
                          p = psw.tile([128, 512], DT.float32, tag="w",
                                       name=f"wo{t}{wlc}")
                          mm(p, wot[:, 0, osl], os_[0][:, wsl], True, False)
                          mm(p, wot[:, 1, osl], os_[1][:, wsl], False, True)
                          hsl = slice(PAD + wlc * 512, PAD + (wlc + 1) * 512)
                          if wlc < NCH - 1:
                              nc.vector.scalar_tensor_tensor(
                                  out=h2p[t][:, hsl], in0=p,
                                  scalar=bof[:, t:t + 1], in1=hp[t][:, hsl],
                                  op0=ALU.add, op1=ALU.add)
                          else:
                              tmpw = tmpp.tile([128, 512], DT.float32, tag="bcs",
                                               name=f"won{t}")
                              nc.vector.tensor_tensor(out=tmpw, in0=p,
                                                      in1=last_bcs, op=ALU.mult)
                              nc.vector.scalar_tensor_tensor(
                                  out=h2p[t][:, hsl], in0=tmpw,
                                  scalar=bof[:, t:t + 1], in1=hp[t][:, hsl],
                                  op0=ALU.add, op1=ALU.add)

                  if lc > 0:
                      wo_chunk(lc - 1)
              wo_chunk(NCH - 1)

            # ---------- conv2 + IN, residual conv on x, leaky, store ----------
            with tc.tile_pool(name="psC", bufs=8, space="PSUM") as psC:
                for t in range(4):
                    osl = slice(t * 128, (t + 1) * 128)
                    st = statp.tile([128, NCH, 6], DT.float32, tag="st2")
                    chunks = []
                    for lc in range(NCH):
                        p = psC.tile([128, 512], DT.float32, tag="c",
                                     name=f"c2p{t}{lc}")
                        n = 0
                        for i in range(2):
                            for k in range(KW):
                                mm(p, w2t[:, i, k, osl],
                                   h2p[i][:, lc * 512 + k: lc * 512 + k + 512],
                                   n == 0, n == 9)
                                n += 1
                        nc.vector.bn_stats(out=st[:, lc, :], in_=p)
                        chunks.append(p)
                    mv = smallp.tile([128, 2], DT.float32, tag="mv")
                    rstd = smallp.tile([128, 1], DT.float32, tag="rstd")
                    negm = smallp.tile([128, 1], DT.float32, tag="negm")
                    nc.vector.bn_aggr(out=mv, in_=st)
                    nc.scalar.activation(out=rstd, in_=mv[:, 1:2], func=AF.Sqrt,
                                         bias=eps_t, scale=1.0)
                    nc.vector.reciprocal(out=rstd, in_=rstd)
                    nc.vector.tensor_scalar(out=negm, in0=mv[:, 0:1], scalar1=rstd,
                                            scalar2=-1.0, op0=ALU.mult, op1=ALU.mult)
                    nsub = 1 if t == 3 else 2
                    W = 512 // nsub
                    for lc in range(NCH):
                        pres = psC.tile([128, 512], DT.float32, tag="c",
                                        name=f"pres{t}{lc}")
                        mm(pres, wrt[:, 0, osl],
                           xp[0][:, PAD + lc * 512:PAD + lc * 512 + 512], True, False)
                        mm(pres, wrt[:, 1, osl],
                           xp[1][:, PAD + lc * 512:PAD + lc * 512 + 512], False, True)
                        for s in range(nsub):
                            lsl = slice(lc * 512 + s * W, lc * 512 + (s + 1) * W)
                            ssl = slice(s * W, (s + 1) * W)
                            j = lc * nsub + s
                            tmp = tmpp.tile([128, W], DT.float32, tag="tmp",
                                            name=f"tmp{t}{j}")
                            nc.scalar.activation(out=tmp, in_=chunks[lc][:, ssl],
                                                 func=AF.Identity, bias=negm,
                                                 scale=rstd)
                            nc.vector.scalar_tensor_tensor(
                                out=tmp, in0=tmp, scalar=brf[:, t:t + 1],
                                in1=pres[:, ssl], op0=ALU.add, op1=ALU.add)
                            oc = outp.tile([128, W], DT.float32, tag="oc",
                                           name=f"oc{t}{j}")
                            nc.vector.scalar_tensor_tensor(out=oc, in0=tmp,
                                                           scalar=SLOPE, in1=tmp,
                                                           op0=ALU.mult, op1=ALU.max)
                            eng = nc.sync if j % 2 == 0 else nc.scalar
                            eng.dma_start(out=out_d[osl, lsl], in_=oc)
    nc.finalize()
    return nc


def _get_nc():
    global _CACHED_NC
    if _CACHED_NC is None:
        _CACHED_NC = _build()
    return _CACHED_NC


def _pack_weights(inputs):
    f = np.float32
    pack = np.zeros((128, F_PACK), dtype=np.float32)

    def put2(name, w):  # w: [256, ...] -> [128, 2*rest], i-major per partition
        a, b = _SEG[name]
        r = w.reshape(2, 128, -1).transpose(1, 0, 2).reshape(128, -1)
        pack[:, a:b] = r

    put2("w2t", inputs["w2"].astype(f).transpose(1, 2, 0))
    put2("wqt", inputs["wq"][:, :, 0].astype(f).T)             # [I,O]
    put2("wkt", inputs["wk"][:, :, 0].astype(f).T)
    put2("wvt", inputs["wv"][:, :, 0].astype(f).T)
    put2("wot", inputs["wo"][:, :, 0].astype(f).T)
    put2("wrt", inputs["wr"][:, :, 0].astype(f).T)
    a, b = _SEG["ones_col"]
    pack[:, a:b] = 1.0
    a, b = _SEG["bq_col"]
    pack[0:CR, a] = inputs["bq"].astype(f)
    a, b = _SEG["bk_col"]
    pack[0:CR, a] = inputs["bk"].astype(f)
    a, b = _SEG["bo_col"]
    pack[:, a:b] = inputs["bo"].astype(f).reshape(2, 128).T
    a, b = _SEG["br_col"]
    pack[:, a:b] = inputs["br"].astype(f).reshape(4, 128).T
    a, b = _SEG["bv"]
    pack[0, a:b] = inputs["bv"].astype(f)
    a, b = _SEG["ones_row"]
    pack[0, a:b] = 1.0
    return pack.astype(BF16)


def _pack_w1(inputs):
    w = inputs["w1"].astype(np.float32).transpose(1, 2, 0)     # [I,K,O]
    return w.reshape(2, 128, -1).transpose(1, 0, 2).reshape(128, -1).astype(BF16)


def _prep_in_maps(inputs):
    wpack = _pack_weights(inputs)
    w1pack = _pack_w1(inputs)
    x = np.asarray(inputs["x"], dtype=np.float32)
    xpad = np.pad(x, ((0, 0), (0, 0), (PAD, PAD)))              # [B, 256, LP]
    xpad = xpad.reshape(B, 2, 128, LP).transpose(0, 2, 1, 3)    # [B, 128, 2, LP]
    return [{"wpack": wpack, "w1pack": w1pack,
             "x": np.ascontiguousarray(xpad[b]).astype(BF16)}
            for b in range(B)]


def run(inputs, trace=False):
    nc = _get_nc()
    in_maps = _prep_in_maps(inputs)
    res = run_bass_kernel_spmd(nc, in_maps, core_ids=list(range(B)), trace=trace)
    out = np.stack([np.asarray(res.results[b]["out"]) for b in range(B)], axis=0)
    return out, res.exec_time_ns


def kernel(**inputs):
    return run(inputs)[0]

